# revision 27
# baseline (speedup 1.0000x reference)
"""Per-graph KNN (K=50) + edge distances + Gaussian RDF on 8 Trainium2 cores.

Strategy (data-parallel over graphs, no cross-core communication):
  - Host: groups the sorted-by-graph nodes into per-core contiguous ranges of
    whole graphs, packs consecutive whole graphs into 128-row tiles, and builds
    per-tile matmul features.
  - Device (per tile): one fp32 matmul produces vals = -(d2 + BG*(b_r-b_c)^2)
    for all 128x128 in-tile pairs (the graph mask rides the contraction with
    exact power-of-two cancellation), the diagonal is killed with
    affine_select, and 7 rounds of max8/max_index/match_replace extract the 56
    smallest distances per row in order. ACT computes sqrt and the 5-bin
    Gaussian RDF; DVE converts local column indices to global node ids.
  - Host: reassembles full outputs, then repairs the few entries where fp32
    rounding on the PE could disagree with the XLA-CPU reference ordering:
    near-tie clusters are re-sorted with a bit-exact XLA-CPU emulation of d2,
    small graphs (<51 nodes) get their BIG-sentinel filler entries, and tiny
    distances are recomputed exactly.
"""
import numpy as np

N = 16384
NUM_GRAPHS = 256
K = 50
NR = 56           # 7 rounds x 8 selected per row
FCOLS = 56 + 50 + 250   # packed float output: V | dist | rdf
ROUNDS = 7
NUM_BINS = 5
RDF_CUTOFF = 10.0
BIG = 1e10
BG = np.float32(8192.0)   # power-of-two graph-mask weight (> max same-graph d2)
NEG_HUGE = -3.0e38
TILE_P = 128
N_CORES = 8
CLUSTER_W = 5e-3          # near-tie window on d2 (device-vs-XLA error << this)
SMALL_DIST = 0.35         # below this, recompute dist/rdf exactly on host

f32 = np.float32
f64 = np.float64

_PROGRAM_CACHE = {}
TRACE = False          # set True (e.g. from test.py) to capture an NTFF trace
LAST_RESULTS = None    # BassKernelResults of the most recent run
SIM_NUMPY = False      # emulate the device in numpy (host-logic dry runs)


def _fma32(a, b, c):
    return np.float32(f64(a) * f64(b) + f64(c))


def _centers():
    return np.linspace(0.0, RDF_CUTOFF, NUM_BINS).astype(f32)


def _gamma():
    spacing = RDF_CUTOFF / (NUM_BINS - 1)
    return f32(1.0 / (2.0 * spacing * spacing))


def _sq_like_reference(pos):
    """sum(pos*pos, -1) computed op-by-op on the XLA CPU backend, bit-exact to
    the reference's value."""
    import jax
    import jax.numpy as jnp
    cpu = jax.devices("cpu")[0]
    with jax.default_device(cpu):
        p = jnp.asarray(pos)
        sq = jnp.sum(p * p, axis=-1)
        return np.asarray(sq)


def _d2_xla(pos, sq, r, c):
    """Bit-exact emulation of the reference's fp32 d2[r, c] on XLA CPU:
    fl(fl(sq_r + sq_c) - 2*fma(z,z', fma(y,y', fl(x*x'))))."""
    xx = (pos[r, 0] * pos[c, 0]).astype(f32)
    M = _fma32(pos[r, 2], pos[c, 2], _fma32(pos[r, 1], pos[c, 1], xx))
    t = (sq[r] + sq[c]).astype(f32)
    return (t - (f32(2.0) * M)).astype(f32)


def _partition(batch):
    """Graph boundaries -> first-fit-decreasing bin packing of whole graphs
    into 128-row tiles -> tiles distributed round-robin across cores.

    A tile is a list of segments (global_start, nrows, local_offset): whole
    graphs placed back to back. Graphs in one tile need not be consecutive —
    the per-row global base (global_start - local_offset) restores global
    column indices, and the graph-id mask keeps rows from matching the other
    graph's columns."""
    sizes = np.bincount(batch, minlength=NUM_GRAPHS)
    assert sizes.max() <= TILE_P, f"graph larger than tile: {sizes.max()}"
    starts = np.concatenate([[0], np.cumsum(sizes)])

    order = sorted(range(NUM_GRAPHS), key=lambda g: -sizes[g])
    bins = []  # list of [fill, [graphs]]
    for g in order:
        m = int(sizes[g])
        if m == 0:
            continue
        placed = False
        for b in bins:
            if b[0] + m <= TILE_P:
                b[0] += m
                b[1].append(g)
                placed = True
                break
        if not placed:
            bins.append([m, [g]])

    # distribute bins across cores, largest-fill first, round robin: slot t
    # then holds similar fills on every core, so the program can scan only
    # F_slots[t] candidate columns at slot t
    bins.sort(key=lambda b: -b[0])
    core_tiles = [[] for _ in range(N_CORES)]
    for i, b in enumerate(bins):
        tile = []
        off = 0
        for g in b[1]:
            tile.append((int(starts[g]), int(sizes[g]), off))
            off += int(sizes[g])
        core_tiles[i % N_CORES].append(tile)
    T = max(len(t) for t in core_tiles)
    F_slots = []
    for t in range(T):
        fill = max(sum(s[1] for s in tiles[t]) if t < len(tiles) else 0
                   for tiles in core_tiles)
        F_slots.append(min(TILE_P, ((fill + 7) // 8) * 8))
    return sizes, starts, core_tiles, tuple(F_slots)


def _build_features(pos, batch, sq, core_tiles, T):
    """Per-core packed inputs: feats [T, 8, 256] f32, base [T, 128, 1] f32.
    Graph ids are remapped to small ints (segment index) within each tile so
    all mask features are bf16-exact: the power-of-two cancellation on the PE
    is exact even if the fp32 matmul decomposes its operands."""
    feats_all, base_all = [], []
    for tiles in core_tiles:
        feats = np.zeros((T, 8, 2 * TILE_P), f32)
        basea = np.zeros((TILE_P, T), f32)
        for t, segs in enumerate(tiles):
            bf = np.full(TILE_P, -1.0, f32)
            x = np.zeros(TILE_P, f32); y = np.zeros(TILE_P, f32)
            z = np.zeros(TILE_P, f32); s = np.zeros(TILE_P, f32)
            for si, (a, n, off) in enumerate(segs):
                bf[off:off + n] = float(si)
                x[off:off + n] = pos[a:a + n, 0]
                y[off:off + n] = pos[a:a + n, 1]
                z[off:off + n] = pos[a:a + n, 2]
                s[off:off + n] = sq[a:a + n]
                basea[off:off + n, t] = float(a - off)
            featA = np.stack([
                -BG * bf * bf, 2 * BG * bf, np.full(TILE_P, -BG, f32),
                -s, np.full(TILE_P, -1.0, f32), 2 * x, 2 * y, 2 * z,
            ]).astype(f32)
            featB = np.stack([
                np.ones(TILE_P, f32), bf, bf * bf,
                np.ones(TILE_P, f32), s, x, y, z,
            ]).astype(f32)
            feats[t, :, :TILE_P] = featA
            feats[t, :, TILE_P:] = featB
        feats_all.append(np.ascontiguousarray(
            feats.transpose(1, 0, 2).reshape(8, T * 2 * TILE_P)))
        base_all.append(basea)
    return feats_all, base_all


def _build_program(F_slots):
    if F_slots in _PROGRAM_CACHE:
        return _PROGRAM_CACHE[F_slots]
    T = len(F_slots)
    import concourse.bacc as bacc
    import concourse.mybir as mybir
    from concourse import tile

    AF = mybir.ActivationFunctionType
    dt = mybir.dt

    class _OneActSetBacc(bacc.Bacc):
        """Force every activation onto the natural_log_exp_and_others table
        set (it contains all funcs used here: Copy/Ln/Exp/Square/Identity) so
        the ACT engine loads its function table exactly once instead of
        thrashing between per-func default sets."""

        def insert_act_table_loads(self):
            from concourse.hw_specs import get_activation_tables
            import bass_rust as _bass_rust
            has_activation = any(
                isinstance(i, mybir.InstActivation)
                for b in self.main_func.blocks
                for i in b.instructions
            )
            if not has_activation:
                return
            want = "natural_log_exp_and_others"
            tables = [
                (name, funcs if name == want else set())
                for name, funcs in get_activation_tables(self.m.arch).items()
            ]
            _bass_rust.insert_act_table_loads(self, tables)

    nc = _OneActSetBacc("TRN2", target_bir_lowering=False, debug=False,
                        num_devices=N_CORES)
    # single batched inputs: feats [8, T*256] (per-tile 256-col blocks),
    # bases [128, T]; packed outputs: floats [T*128, 356] = V(56) | dist(50)
    # | rdf(250), ints [T*128, 56] = global src indices
    feats_d = nc.declare_dram_parameter("feats", [8, T * 2 * TILE_P], dt.float32, isOutput=False)
    base_d = nc.declare_dram_parameter("base", [TILE_P, T], dt.float32, isOutput=False)
    outi_o = nc.declare_dram_parameter("outi_o", [T * TILE_P, NR], dt.int32, isOutput=True)
    outf_o = nc.declare_dram_parameter("outf_o", [T * TILE_P, FCOLS], dt.float32, isOutput=True)

    centers = _centers()
    gamma = float(_gamma())

    with tile.TileContext(nc) as tc:
        with (
            tc.tile_pool(name="cst", bufs=1) as cst,
            tc.tile_pool(name="sb", bufs=3) as sb,
            tc.tile_pool(name="sel", bufs=6) as sel,
            tc.tile_pool(name="vpool", bufs=1) as vpool,
            tc.tile_pool(name="ps", bufs=4, space="PSUM") as ps,
        ):
            # centers constant, repeated per edge slot: [c0..c4] * K
            ctile = cst.tile([TILE_P, K * NUM_BINS], dt.float32, tag="ctile")
            for j in range(NUM_BINS):
                nc.gpsimd.memset(ctile[:, j::NUM_BINS], float(centers[j]))

            # single input DMAs for every tile's features and bases
            fAll = cst.tile([8, T * 2 * TILE_P], dt.float32, tag="fAll")
            bAll = cst.tile([TILE_P, T], dt.float32, tag="bAll")
            nc.sync.dma_start(out=fAll[:, :2 * TILE_P],
                              in_=feats_d[:, :2 * TILE_P])
            nc.sync.dma_start(out=fAll[:, 2 * TILE_P:],
                              in_=feats_d[:, 2 * TILE_P:])
            nc.sync.dma_start(out=bAll[:, :], in_=base_d[:, :])

            # pass A0: all distance matrices into SBUF (PE/ACT/Pool only) so
            # the DVE top-k stream below never waits on a fresh tile
            vals_tiles = []
            for t in range(T):
                F = F_slots[t]
                c0 = t * 2 * TILE_P
                acc = ps.tile([TILE_P, F], dt.float32, tag="acc")
                nc.tensor.matmul(acc[:, :], fAll[:, c0:c0 + TILE_P],
                                 fAll[:, c0 + TILE_P:c0 + TILE_P + F],
                                 start=True, stop=True)

                vals = vpool.tile([TILE_P, F], dt.float32, tag=f"vals{t}")
                nc.scalar.copy(out=vals[:, :], in_=acc[:, :])
                # kill the self-match diagonal: iota = col - row
                nc.gpsimd.affine_select(
                    out=vals[:, :], in_=vals[:, :], pattern=[[1, F]],
                    compare_op=mybir.AluOpType.not_equal, fill=NEG_HUGE,
                    base=0, channel_multiplier=-1)
                vals_tiles.append(vals)

            # pass A: top-k + sqrt + rdf; the rdf tail of tile t-1 is
            # emitted after tile t's dist ops so ACT never stalls mid-chain
            # waiting for the Pool diff of the same tile
            pending_rdf = None

            def emit_rdf(dist_ap, rows_):
                diff = sel.tile([TILE_P, K * NUM_BINS], dt.float32, tag="dsq")
                nc.gpsimd.tensor_sub(
                    out=diff[:, :].rearrange("p (a b) -> p a b", b=NUM_BINS),
                    in0=dist_ap.to_broadcast([TILE_P, K, NUM_BINS]),
                    in1=ctile[:, :].rearrange("p (a b) -> p a b", b=NUM_BINS))
                nc.scalar.activation(out=diff[:, :], in_=diff[:, :],
                                     func=AF.Square, bias=0.0, scale=1.0)
                rows_sl, rdf_ap, vf_t = rows_
                nc.scalar.activation(out=rdf_ap, in_=diff[:, :],
                                     func=AF.Exp, bias=0.0, scale=-gamma)
                nc.sync.dma_start(out=outf_o[rows_sl, :], in_=vf_t[:, :])

            for t in range(T):
                vals = vals_tiles[t]
                bs = bAll[:, t:t + 1]
                vf = sel.tile([TILE_P, FCOLS], dt.float32, tag="vf")
                V = vf[:, 0:NR]
                I = sel.tile([TILE_P, NR], dt.uint32, tag="I")
                for r in range(ROUNDS):
                    v8 = V[:, r * 8:(r + 1) * 8]
                    i8 = I[:, r * 8:(r + 1) * 8]
                    nc.vector.max(out=v8, in_=vals[:, :])
                    nc.vector.max_index(out=i8, in_max=v8, in_values=vals[:, :])
                    if r + 1 < ROUNDS:  # last round needs no replace
                        nc.vector.match_replace(out=vals[:, :],
                                                in_to_replace=v8,
                                                in_values=vals[:, :],
                                                imm_value=NEG_HUGE)

                rows = slice(t * TILE_P, (t + 1) * TILE_P)
                if pending_rdf is not None:
                    emit_rdf(pending_rdf[0], pending_rdf[1])
                    pending_rdf = None
                src = sel.tile([TILE_P, NR], dt.int32, tag="src")
                nc.gpsimd.tensor_scalar(out=src[:, :], in0=I[:, :],
                                        scalar1=bs, scalar2=None,
                                        op0=mybir.AluOpType.add)
                nc.sync.dma_start(out=outi_o[rows, :], in_=src[:, :])

                # clamp vals to <= -1e-12 so log never sees a negative d2;
                # sqrt(d2) = exp(0.5*ln(d2)) keeps every ACT func in the
                # natural_log_exp_and_others table set (no per-tile reloads)
                vc = sel.tile([TILE_P, K], dt.float32, tag="vc")
                nc.gpsimd.tensor_scalar_min(vc[:, :], V[:, :K], -1e-12)
                lg = sel.tile([TILE_P, K], dt.float32, tag="lg")
                nc.scalar.activation(out=lg[:, :], in_=vc[:, :],
                                     func=AF.Ln, bias=0.0, scale=-1.0)
                dist = vf[:, NR:NR + K]
                nc.scalar.activation(out=dist, in_=lg[:, :],
                                     func=AF.Exp, bias=0.0, scale=0.5)

                pending_rdf = (dist, (rows, vf[:, NR + K:FCOLS], vf))

            emit_rdf(pending_rdf[0], pending_rdf[1])

    nc.compile()
    _PROGRAM_CACHE[F_slots] = nc
    return nc


class _FakeResults:
    def __init__(self, results):
        self.results = results
        self.exec_time_ns = None
        self.mean_exec_time_ns = None
        self.instructions_and_trace = None
        self.max_exec_time_core_id = None


def _numpy_device(in_maps, T):
    """Numpy emulation of the device program (sequential-fp32 matmul MACs,
    max8-style selection) for host-logic dry runs."""
    centers = _centers()
    gamma = _gamma()
    results = []
    for m in in_maps:
        feats = m["feats"]; base = m["base"]
        outi = np.zeros((T * TILE_P, NR), np.int32)
        outf = np.zeros((T * TILE_P, FCOLS), f32)
        for t in range(T):
            fA = feats[:, t * 2 * TILE_P:t * 2 * TILE_P + TILE_P]
            fB = feats[:, t * 2 * TILE_P + TILE_P:(t + 1) * 2 * TILE_P]
            acc = np.zeros((TILE_P, TILE_P), f32)
            for k in range(8):
                acc = (acc + (fA[k][:, None] * fB[k][None, :]).astype(f32)).astype(f32)
            np.fill_diagonal(acc, f32(NEG_HUGE))
            order = np.argsort(-acc.astype(np.float64), axis=1, kind="stable")[:, :NR]
            V = np.take_along_axis(acc, order, axis=1)
            rows = slice(t * TILE_P, (t + 1) * TILE_P)
            outf[rows, :NR] = V
            outi[rows] = order + base[:, t].astype(np.int64)[:, None]
            d2c = np.maximum(-V[:, :K], f32(1e-12))
            d = np.sqrt(d2c).astype(f32)
            outf[rows, NR:NR + K] = d
            df = (d[:, :, None] - centers[None, None, :]).astype(f32)
            outf[rows, NR + K:] = np.exp((-gamma * (df * df).astype(f32))).astype(f32).reshape(TILE_P, -1)
        results.append({"outi_o": outi, "outf_o": outf})
    return _FakeResults(results)


def _filler_indices(batch, starts, row, need):
    """First `need` global indices outside `row`'s graph (plus `row` itself),
    ascending — the reference's BIG-sentinel tie-break order."""
    g = batch[row]
    a, b = int(starts[g]), int(starts[g + 1])
    out = []
    j = 0
    while len(out) < need:
        if j == a:
            if row < b:  # self is invalid too
                out.append(row)
                if len(out) >= need:
                    break
            j = b
            continue
        out.append(j)
        j += 1
    # the sequence above appends `row` at position when j hits the graph; but
    # ordering must be ascending overall: row >= a > previous appends, and the
    # remaining appends are >= b > row only if row < b. Fix ordering robustly:
    out = sorted(out)[:need]
    return out


def kernel(pos, batch):
    pos = np.ascontiguousarray(np.asarray(pos), dtype=np.float32)
    batch = np.asarray(batch).astype(np.int64)
    assert pos.shape == (N, 3) and batch.shape == (N,)

    from concourse.bass_utils import run_bass_kernel_spmd

    sq = _sq_like_reference(pos)
    sizes, starts, core_tiles, F_slots = _partition(batch)
    T = len(F_slots)
    feats_all, base_all = _build_features(pos, batch, sq, core_tiles, T)

    in_maps = [{"feats": feats_all[c], "base": base_all[c]}
               for c in range(N_CORES)]
    if SIM_NUMPY:
        res = _numpy_device(in_maps, T)
    else:
        nc = _build_program(F_slots)
        if TRACE:
            try:
                res = run_bass_kernel_spmd(nc, in_maps, list(range(N_CORES)),
                                           trace=True)
            except ModuleNotFoundError:
                res = run_bass_kernel_spmd(nc, in_maps, list(range(N_CORES)))
        else:
            res = run_bass_kernel_spmd(nc, in_maps, list(range(N_CORES)))
    global LAST_RESULTS
    LAST_RESULTS = res

    # ---- reassemble ----
    src56 = np.zeros((N, NR), np.int64)
    d2dev = np.full((N, NR), np.inf, f32)
    dist = np.zeros((N, K), f32)
    rdf = np.zeros((N, K, NUM_BINS), f32)
    for c in range(N_CORES):
        out = res.results[c]
        s_o = out["outi_o"]; f_o = out["outf_o"]
        for t, segs in enumerate(core_tiles[c]):
            for (a, n, off) in segs:
                rs = slice(t * TILE_P + off, t * TILE_P + off + n)
                gl = slice(a, a + n)
                src56[gl] = s_o[rs]
                d2dev[gl] = -f_o[rs, :NR]
                dist[gl] = f_o[rs, NR:NR + K]
                rdf[gl] = f_o[rs, NR + K:].reshape(n, K, NUM_BINS)

    centers = _centers()
    gamma = _gamma()

    def exact_fix(rows, cols):
        """Overwrite dist/rdf at (row, k) positions with exact host values."""
        if len(rows) == 0:
            return
        rows = np.asarray(rows); cols = np.asarray(cols)
        emit = cols < K
        er, ec = rows[emit], cols[emit]
        d2e = _d2_xla(pos, sq, er, src56[er, ec])
        de = np.sqrt(np.maximum(d2e, f32(1e-12))).astype(f32)
        dist[er, ec] = de
        df = (de[:, None] - centers[None, :]).astype(f32)
        rdf[er, ec] = np.exp((-gamma * (df * df).astype(f32))).astype(f32)

    # ---- near-tie cluster repair (exact XLA ordering) ----
    nvalid = np.minimum(NR, sizes[batch] - 1)     # per-row count of real entries
    kk = np.arange(NR)
    validm = kk[None, :] < nvalid[:, None]
    gaps = np.diff(d2dev, axis=1)
    link = (gaps < CLUSTER_W) & validm[:, 1:] & validm[:, :-1]
    rows_with = np.where(link.any(axis=1))[0]
    fix_r, fix_k = [], []
    for i in rows_with:
        li = link[i]
        k = 0
        while k < NR - 1:
            if not li[k]:
                k += 1
                continue
            a = k
            while k < NR - 1 and li[k]:
                k += 1
            b = k  # cluster entries a..b inclusive
            if a >= K:
                continue
            idx = src56[i, a:b + 1]
            d2x = _d2_xla(pos, sq, np.full(idx.shape, i), idx)
            order = np.lexsort((idx, d2x))
            src56[i, a:b + 1] = idx[order]
            for j in range(a, min(b + 1, K)):
                fix_r.append(i); fix_k.append(j)
    exact_fix(fix_r, fix_k)

    # ---- filler entries for graphs with < K+1 nodes ----
    small = np.where(sizes <= K)[0]
    fill_dist = np.sqrt(f32(BIG)).astype(f32)
    dfc = (f32(fill_dist) - centers).astype(f32)
    fill_rdf = np.exp((-gamma * (dfc * dfc).astype(f32))).astype(f32)
    for g in small:
        m = int(sizes[g])
        need = K - (m - 1)
        for i in range(int(starts[g]), int(starts[g + 1])):
            fillers = _filler_indices(batch, starts, i, need)
            src56[i, m - 1:K] = fillers
            dist[i, m - 1:K] = fill_dist
            rdf[i, m - 1:K] = fill_rdf

    # ---- tiny distances: PE noise has bad relative error there ----
    sus = (dist < SMALL_DIST) | ~np.isfinite(dist)
    sus &= (np.arange(K)[None, :] < np.minimum(K, sizes[batch] - 1)[:, None])
    sr, sk = np.where(sus)
    exact_fix(sr, sk)

    src = src56[:, :K].astype(np.int32).reshape(-1)
    dst = np.repeat(np.arange(N, dtype=np.int32), K)
    edge_index = np.stack([src, dst])
    return edge_index, dist.reshape(-1), rdf.reshape(N * K, NUM_BINS)
